# revision 50
# baseline (speedup 1.0000x reference)
"""CPModule (3-axis line-interp product) TRN2 kernel — transfer-optimized.

out[c, n] = prod_a lerp(param_a[c, :], pos_a(n)),  pos = (x+1)*149.5.

Per-axis linear interpolation is a K=128 matmul with a "two-hot" hat-basis
matrix e[g, t] = relu(1 - |pos_t - g|): v_a = P_a @ e_a.  The 300-row grid is
split into 3 overlapping 128-row chunks at stride 127; unlike the v1 kernel
(which bucket-sorted points on host so each group touched one chunk), every
point's hat weights are computed for ALL THREE chunks and the three partial
products are accumulated in PSUM.  Grid rows duplicated between chunks (127,
254) are zeroed in the later chunk's table so the sum is exact.  This makes
the program input-independent: no host argsort, no unpermute, and the jitted
shard_map executable is built once per process and cached — warm calls only
transfer inputs, run, and fetch outputs.

The dominant cost is the ~50-70 MB/s axon tunnel.  The [48, 2M] f32 output
(384 MB, ~5.5 s) is therefore returned as int8 with a per-(comp row,
512-point half-group) scale and dequantized on the host; the f32 scales ride
along in the last 1960 columns of the one int8 output tensor (one fetch, ~96
MB total).  Quantization is uniform with step absmax_row/126.5, so the error
is <= absmax/253 (~0.4% of global absmax), far under the 2e-2 gate.  Coords
are shipped as int16 (round(x*32767), 12 MB instead of 24 MB; adds ~0.4% more
error, still 2x under the gate) and widened on device.  Donated output
buffers are created device-side (no 100-400 MB host->device zeros).  Output
shards are fetched one at a time while worker threads dequantize the
previous core's block.  A 7-bit packed variant (8 values -> 7 bytes) was
measured: the 12% byte cut is swamped by tunnel variance and the host-side
unpack cost; int8 wins.

The call runs as N_HALF=2 sequential executions of one half-size program:
the tunnel is duplex enough (measured: a concurrent 12 MB upload adds ~0.2 s
to a 96 MB fetch instead of ~0.4 s serial) that half 2's coordinate upload,
zero-buffer memset, and exec all hide under half 1's output fetch, and the
first fetch starts ~0.2 s earlier.  Going to 4 splits is a wash: each extra
split adds ~8 per-shard fetch round-trips (~10 ms each) for less head saving.

Device pipeline per group (1024 pts = 2 column-tiles of 512 packed into
psum rows [0:64) and [64:128)):
  PE:   broadcast coord row -> psum bc [128, 1024] (K=1 matmul with ones)
        per chunk c: v matmuls [48->64, 512] accumulate into vp psum
  ACT:  t_c = |149.5*x + (149.5 - 127c - lane)|   (abs pass, psum -> sbuf)
  DVE:  e'_c = min(t_c, 1) - 1  (= -relu(1-|.|); tables are negated)
        out = v0 * v1 * v2, absmax-reduce, reciprocal, int8 quantize
  DMA:  out tile [48, 512] x2 -> HBM int8, per-group scales at the end
"""

import sys

sys.path.insert(0, "/opt/trn_rl_repo")

import contextlib
import os

os.environ.setdefault("JAX_PLATFORMS", "axon,cpu")

import numpy as np

import concourse.bass as bass
import concourse.mybir as mybir
from concourse import tile

F32 = mybir.dt.float32
I8 = mybir.dt.int8
I16 = mybir.dt.int16
AF = mybir.ActivationFunctionType
ALU = mybir.AluOpType

N_COMP = 48
G = 300
N_CORES = 8
TILE = 512
GROUP = 2 * TILE  # 1024 points per device group
N_PTS = 2_000_000
NPC = N_PTS // N_CORES  # 250_000 points per core
# The work is split into N_HALF sequential executions of one half-size
# program: the axon tunnel is duplex enough that half 2's coord upload and
# exec hide under half 1's output fetch.
N_HALF = 2
NPC_H = NPC // N_HALF  # 125_000 points per core per half
N_GROUPS = -(-NPC_H // GROUP)  # 123
NPAD = N_GROUPS * GROUP  # 125_952
SLAB = 8  # groups of coords per load slab
QMAX = 126.5  # quant range; <127 so rounding can't overflow int8
CSCALE = 32767.0  # coords are shipped as int16 = round(x * CSCALE)
# the [2*48, N_GROUPS] f32 scales ride along in the int8 output tensor:
# 2*48*N_GROUPS*4 bytes spread over 48 rows of extra int8 columns
SCL_COLS = 2 * N_GROUPS * 4  # 984
NCOL = NPAD + SCL_COLS


def _legalize_sync_waits(nc, max_waits=1):
    """This walrus build accepts at most one sync-wait per instruction; split
    extra waits onto preceding same-engine drains (same-queue => in order)."""
    n = 0
    for f in nc.m.functions:
        for bb in f.blocks:
            new_list = []
            for ins in bb.instructions:
                si = ins.sync_info
                waits = list(si.on_wait) if si and si.on_wait else []
                if len(waits) > max_waits:
                    head, tail = waits[:-max_waits], waits[-max_waits:]
                    for w in head:
                        n += 1
                        import bass_rust as _br
                        new_list.append(
                            _br.InstNoOp(
                                name=f"{ins.name}-wsplit-{n}",
                                engine=ins.engine,
                                ins=[],
                                outs=[],
                                sync_info=mybir.SyncInfo(on_wait=[w], on_update=[]),
                            )
                        )
                    ins.sync_info = mybir.SyncInfo(
                        on_wait=tail,
                        on_update=list(si.on_update) if si.on_update else [],
                    )
                new_list.append(ins)
            bb.instructions[:] = new_list
    return n


def _build_program():
    nc = bass.Bass("TRN2", target_bir_lowering=False, debug=False, num_devices=N_CORES)
    d_coords = nc.dram_tensor("coords", [3, NPAD], I16, kind="ExternalInput")
    d_lhsT = nc.dram_tensor("lhsT", [9, 128, 64], F32, kind="ExternalInput")
    d_bias = nc.dram_tensor("bias", [128, 3], F32, kind="ExternalInput")
    d_outq = nc.dram_tensor("outq", [N_COMP, NCOL], I8, kind="ExternalOutput")

    with tile.TileContext(nc) as tc:
        with contextlib.ExitStack() as ctx:
            const = ctx.enter_context(tc.tile_pool(name="const", bufs=1))
            slabp = ctx.enter_context(tc.tile_pool(name="slabp", bufs=2))
            work = ctx.enter_context(tc.tile_pool(name="work", bufs=2))
            outp = ctx.enter_context(tc.tile_pool(name="outp", bufs=3))
            qp = ctx.enter_context(tc.tile_pool(name="qp", bufs=3))
            bcp = ctx.enter_context(tc.tile_pool(name="bcp", bufs=1, space="PSUM"))
            vpp = ctx.enter_context(tc.tile_pool(name="vpp", bufs=6, space="PSUM"))

            lhsT = const.tile([128, 9 * 64], F32)
            nc.sync.dma_start(
                lhsT[:].rearrange("p (n d) -> p n d", d=64),
                d_lhsT.ap().rearrange("n p d -> p n d"),
            )
            biast = const.tile([128, 3], F32)
            nc.sync.dma_start(biast[:], d_bias.ap())
            onest = const.tile([65, 128], F32)
            for a in range(3):
                nc.vector.memset(onest[32 * a : 32 * a + 1, :], 1.0)
            scl = const.tile([128, N_GROUPS], F32)

            conv = None
            for g in range(N_GROUPS):
                s = g % SLAB
                if s == 0:
                    ncols = min(SLAB * GROUP, NPAD - g * GROUP)
                    slab = slabp.tile([65, SLAB * GROUP], I16, name="slab", tag="slab")
                    for a in range(3):
                        nc.sync.dma_start(
                            slab[32 * a : 32 * a + 1, 0:ncols],
                            d_coords.ap()[a : a + 1, g * GROUP : g * GROUP + ncols],
                        )
                    conv = slabp.tile([65, SLAB * GROUP], F32, name="conv", tag="conv")
                    for a in range(3):
                        nc.vector.tensor_copy(
                            conv[32 * a : 32 * a + 1, 0:ncols],
                            slab[32 * a : 32 * a + 1, 0:ncols],
                        )
                vps = []
                for a in range(3):
                    crow = conv[32 * a : 32 * a + 1, s * GROUP : (s + 1) * GROUP]
                    bc = bcp.tile([128, GROUP], F32, name=f"bc_{g}_{a}", tag="bc")
                    nc.tensor.matmul(
                        bc[:, 0:TILE], onest[32 * a : 32 * a + 1, :], crow[:, 0:TILE],
                        start=True, stop=True,
                    )
                    nc.tensor.matmul(
                        bc[:, TILE:GROUP], onest[32 * a : 32 * a + 1, :], crow[:, TILE:GROUP],
                        start=True, stop=True,
                    )
                    vp = vpp.tile([128, TILE], F32, name=f"vp_{g}_{a}", tag="vp")
                    for c in range(3):
                        tabs = work.tile(
                            [128, GROUP], F32, name=f"tabs_{g}_{a}_{c}", tag="tabs", bufs=3
                        )
                        nc.scalar.activation(
                            tabs[:], bc[:], AF.Abs, bias=biast[:, c : c + 1],
                            scale=float(149.5 / CSCALE),
                        )
                        eneg = work.tile(
                            [128, GROUP], F32, name=f"eneg_{g}_{a}_{c}", tag="eneg", bufs=3
                        )
                        nc.vector.tensor_scalar(
                            eneg[:], tabs[:], 1.0, 1.0, ALU.min, ALU.subtract
                        )
                        lt = lhsT[:, (a * 3 + c) * 64 : (a * 3 + c + 1) * 64]
                        nc.tensor.matmul(
                            vp[0:64, :], lt, eneg[:, 0:TILE],
                            start=(c == 0), stop=(c == 2), tile_position=(0, 0),
                        )
                        nc.tensor.matmul(
                            vp[64:128, :], lt, eneg[:, TILE:GROUP],
                            start=(c == 0), stop=(c == 2), tile_position=(0, 64),
                        )
                    vps.append(vp)

                v1sb = outp.tile([128, TILE], F32, name=f"v1sb_{g}", tag="v1sb")
                nc.vector.tensor_copy(v1sb[:], vps[1][:])
                p01 = outp.tile([128, TILE], F32, name=f"p01_{g}", tag="p01")
                nc.vector.tensor_mul(p01[:], vps[0][:], v1sb[:])
                outt = outp.tile([128, TILE], F32, name=f"outt_{g}", tag="outt")
                nc.vector.tensor_mul(outt[:], vps[2][:], p01[:])

                nc.vector.tensor_reduce(
                    scl[:, g : g + 1], outt[:], axis=mybir.AxisListType.X,
                    op=ALU.max, apply_absolute_value=True,
                )
                clamped = qp.tile([128, 1], F32, name=f"cl_{g}", tag="cl")
                nc.vector.tensor_scalar_max(clamped[:], scl[:, g : g + 1], 1e-12)
                rcp = qp.tile([128, 1], F32, name=f"rcp_{g}", tag="rcp")
                nc.vector.reciprocal(rcp[:], clamped[:])
                outq = qp.tile([128, TILE], I8, name=f"outq_{g}", tag="outq")
                nc.vector.tensor_scalar(
                    outq[:], outt[:], rcp[:, 0:1], QMAX, ALU.mult, ALU.mult
                )

                off = g * GROUP
                nc.sync.dma_start(d_outq.ap()[:, off : off + TILE], outq[0:N_COMP, :])
                nc.sync.dma_start(
                    d_outq.ap()[:, off + TILE : off + GROUP], outq[64 : 64 + N_COMP, :]
                )

            # scales ride in the last SCL_COLS int8 columns: rows 0:48 are the
            # half-A scales ([48, N_GROUPS] f32 = [48, 4*N_GROUPS] bytes),
            # rows 64:112 the half-B scales
            half = 4 * N_GROUPS
            sclb = scl[:].bitcast(I8)  # [128, 4*N_GROUPS]
            nc.sync.dma_start(
                d_outq.ap()[:, NPAD : NPAD + half], sclb[0:N_COMP, :]
            )
            nc.sync.dma_start(
                d_outq.ap()[:, NPAD + half : NPAD + 2 * half], sclb[64 : 64 + N_COMP, :]
            )

    from concourse.hw_specs import get_activation_tables
    import bass_rust as _br
    _br.insert_act_table_loads(nc, list(get_activation_tables(nc.m.arch).items()))
    _legalize_sync_waits(nc)
    return nc


_RT: dict = {}
_OUT_BUF = None
_TABLES = None
# Device output buffers from the previous call, reused as the donated
# "zero" operands of the next call. The program writes every byte of the
# output tensor (all point columns via the group loop, all scale columns via
# the two trailing DMAs), so zero-init is only a first-call formality and
# skipping the ~50 ms memset+dispatch takes it off the exec critical path.
_PREV: list = []


def _runtime():
    """Build the Bass program and the jitted shard_map executable once."""
    if _RT:
        return _RT
    import jax
    import jax.numpy as jnp
    from jax.experimental.shard_map import shard_map
    from jax.sharding import Mesh, NamedSharding, PartitionSpec as P

    from concourse.bass2jax import (
        _bass_exec_p,
        install_neuronx_cc_hook,
        partition_id_tensor,
    )

    install_neuronx_cc_hook()
    nc = _build_program()

    partition_name = nc.partition_id_tensor.name if nc.partition_id_tensor else None
    in_names, out_names, out_avals = [], [], []
    for alloc in nc.m.functions[0].allocations:
        if not isinstance(alloc, mybir.MemoryLocationSet):
            continue
        name = alloc.memorylocations[0].name
        if alloc.kind == "ExternalInput":
            if name != partition_name:
                in_names.append(name)
        elif alloc.kind == "ExternalOutput":
            out_names.append(name)
            out_avals.append(
                jax.core.ShapedArray(tuple(alloc.tensor_shape), mybir.dt.np(alloc.dtype))
            )
    assert in_names == ["coords", "lhsT", "bias"], in_names
    assert out_names == ["outq"], out_names
    n_params = len(in_names)
    n_outs = len(out_names)
    all_names = in_names + out_names
    if partition_name is not None:
        all_names.append(partition_name)
    all_names = tuple(all_names)

    def _body(*args):
        operands = list(args)
        if partition_name is not None:
            operands.append(partition_id_tensor())
        outs = _bass_exec_p.bind(
            *operands,
            out_avals=tuple(out_avals),
            in_names=all_names,
            out_names=tuple(out_names),
            lowering_input_output_aliases=(),
            sim_require_finite=True,
            sim_require_nnan=True,
            nc=nc,
        )
        return tuple(outs)

    devices = jax.devices()[:N_CORES]
    assert len(devices) == N_CORES
    mesh = Mesh(np.asarray(devices), ("core",))
    sh = NamedSharding(mesh, P("core"))
    donate = tuple(range(n_params, n_params + n_outs))
    sharded = jax.jit(
        shard_map(
            _body,
            mesh=mesh,
            in_specs=(P("core"),) * (n_params + n_outs),
            out_specs=(P("core"),) * n_outs,
            check_rep=False,
        ),
        donate_argnums=donate,
        keep_unused=True,
    )

    # both halves' donated zero buffers in one dispatch
    zeros = jax.jit(
        lambda: tuple(
            jnp.zeros((N_CORES * N_COMP, NCOL), jnp.int8) for _ in range(N_HALF)
        ),
        out_shardings=(sh,) * N_HALF,
    )

    _RT.update(sharded=sharded, zeros=zeros, sh=sh)
    return _RT


def _build_tables(params):
    """lhsT[a*3+c] = -param_a[:, 127c : 127c+128].T zero-padded to [128, 64].
    Lane 0 of chunks 1,2 duplicates lane 127 of the previous chunk (grid rows
    127, 254) — zero it there so summing all three chunk products is exact."""
    lhsT9 = np.zeros((9, 128, 64), dtype=np.float32)
    for a in range(3):
        for c in range(3):
            rows = params[a][:, 127 * c : 127 * c + 128]
            lhsT9[a * 3 + c, : rows.shape[1], :N_COMP] = -rows.T
            if c > 0:
                lhsT9[a * 3 + c, 0, :] = 0.0
    bias = np.zeros((128, 3), dtype=np.float32)
    for c in range(3):
        bias[:, c] = 149.5 - 127.0 * c - np.arange(128)
    return lhsT9, bias


def _dequant_core(k, h, qk, out):
    """Dequantize core k / half h's int8 block [48, NCOL] into
    out[:, k*NPC + h*NPC_H : ...]."""
    ngf = NPC_H // GROUP  # 122 full groups per half
    full = ngf * GROUP
    tail = NPC_H - full
    strided = np.lib.stride_tricks.as_strided
    shalf = 4 * N_GROUPS
    inv = np.float32(1.0 / QMAX)
    sA = qk[:, NPAD : NPAD + shalf].copy().view(np.float32) * inv  # [48, N_GROUPS]
    sB = qk[:, NPAD + shalf :].copy().view(np.float32) * inv
    # scale per (comp, group, half-tile): [48, ngf, 2, 1]
    s3 = np.stack([sA[:, :ngf], sB[:, :ngf]], axis=2)[..., None]
    base = out[:, k * NPC + h * NPC_H :]
    B = strided(base, shape=(N_COMP, ngf, 2, TILE),
                strides=(out.strides[0], GROUP * 4, TILE * 4, 4))
    Q = strided(qk, shape=(N_COMP, ngf, 2, TILE),
                strides=(qk.strides[0], GROUP, TILE, 1))
    np.multiply(Q, s3, out=B)
    if tail:
        np.multiply(qk[:, full : full + tail], sA[:, ngf : ngf + 1],
                    out=base[:, full:NPC_H])


def kernel(xyz_sampled, param0, param1, param2):
    from concurrent.futures import ThreadPoolExecutor

    xyz = np.asarray(xyz_sampled, dtype=np.float32)
    params = [
        np.ascontiguousarray(p.reshape(p.shape[1], p.shape[2]), dtype=np.float32)
        for p in (param0, param1, param2)
    ]
    n = xyz.shape[0]
    assert n == N_PTS and n % N_CORES == 0

    import jax

    rt = _runtime()

    # The tables derive from the tiny params (172 KB) — cache the device
    # arrays keyed on a content hash so identical warm calls skip the rebuild
    # and the ~2.4 MB upload on the exec critical path.
    import hashlib

    global _TABLES
    key = hashlib.blake2b(
        b"".join(p.tobytes() for p in params), digest_size=16
    ).digest()
    if _TABLES is None or _TABLES[0] != key:
        lhsT9, bias = _build_tables(params)
        lhsT_d = jax.device_put(np.tile(lhsT9, (N_CORES, 1, 1)), rt["sh"])
        bias_d = jax.device_put(np.tile(bias, (N_CORES, 1)), rt["sh"])
        _TABLES = (key, lhsT_d, bias_d)
    else:
        _, lhsT_d, bias_d = _TABLES

    global _PREV
    if len(_PREV) == N_HALF:
        zs, _PREV = _PREV, []
    else:
        zs = rt["zeros"]()
    xr = xyz.reshape(N_CORES, N_HALF, NPC_H, 3)
    # dispatch halves back-to-back: half h+1's coord prep and upload plus its
    # exec all overlap half h's upload/exec/output-fetch (the tunnel is
    # duplex); quantize each half's coords just before its dispatch so the
    # later halves' host prep also hides under earlier transfers
    outs = []
    coords = []  # keep buffers alive until fetch completes
    for h in range(N_HALF):
        ch = np.empty((N_CORES, 3, NPAD), dtype=np.int16)
        ch[:, :, NPC_H:] = 0  # pad coords (any in-range value works)
        ch[:, :, :NPC_H] = np.rint(
            xr[:, h] * np.float32(CSCALE)
        ).astype(np.int16).transpose(0, 2, 1)
        coords.append(ch)
        (o,) = rt["sharded"](ch.reshape(N_CORES * 3, NPAD), lhsT_d, bias_d, zs[h])
        outs.append(o)

    shardlists = []
    for o in outs:
        sl = sorted(
            (sd.index[0].start // N_COMP, sd.data) for sd in o.addressable_shards
        )
        for _, d in sl:
            d.copy_to_host_async()
        shardlists.append(sl)

    # Reuse the 384 MB result buffer across calls: a fresh np.empty is backed
    # by new mmap pages, and first-touch page faults during dequant cost
    # ~0.1 s per call. Every byte is rewritten below before return.
    global _OUT_BUF
    if _OUT_BUF is None or _OUT_BUF.shape != (N_COMP, n):
        _OUT_BUF = np.empty((N_COMP, n), dtype=np.float32)
    out = _OUT_BUF
    # Fetch per-shard over the (serial) tunnel; dequantize each core's block
    # in a worker thread while the next shard transfers.
    with ThreadPoolExecutor(4) as ex:
        futs = []
        for h, sl in enumerate(shardlists):
            for k, d in sl:
                qk = np.asarray(d)  # [48, NCOL] int8
                futs.append(ex.submit(_dequant_core, k, h, qk, out))
        for f in futs:
            f.result()
    _PREV = outs  # fully fetched; donate these buffers on the next call
    return out


if __name__ == "__main__":
    # quick self-test against numpy reference on the full-size random input
    rng = np.random.default_rng(0)
    xyz = rng.uniform(-1, 1, size=(N_PTS, 3)).astype(np.float32)
    ps = [0.2 * rng.standard_normal((1, N_COMP, G, 1)).astype(np.float32) for _ in range(3)]

    def ref_interp(p, coord):
        pp = p[0, :, :, 0]
        pos = (coord + 1.0) * 0.5 * (G - 1)
        i0 = np.clip(np.floor(pos).astype(np.int64), 0, G - 1)
        i1 = np.minimum(i0 + 1, G - 1)
        w = (pos - i0).astype(np.float32)
        return pp[:, i0] * (1.0 - w) + pp[:, i1] * w

    got = kernel(xyz, *ps)
    exp = ref_interp(ps[0], xyz[:, 0]) * ref_interp(ps[1], xyz[:, 1]) * ref_interp(ps[2], xyz[:, 2])
    err = np.abs(got - exp).max()
    print("max abs err:", err, "absmax:", np.abs(exp).max(), "rel:", err / np.abs(exp).max())


# revision 52
# speedup vs baseline: 1.0044x; 1.0044x over previous
"""CPModule (3-axis line-interp product) TRN2 kernel — transfer-optimized.

out[c, n] = prod_a lerp(param_a[c, :], pos_a(n)),  pos = (x+1)*149.5.

Per-axis linear interpolation is a K=128 matmul with a "two-hot" hat-basis
matrix e[g, t] = relu(1 - |pos_t - g|): v_a = P_a @ e_a.  The 300-row grid is
split into 3 overlapping 128-row chunks at stride 127; unlike the v1 kernel
(which bucket-sorted points on host so each group touched one chunk), every
point's hat weights are computed for ALL THREE chunks and the three partial
products are accumulated in PSUM.  Grid rows duplicated between chunks (127,
254) are zeroed in the later chunk's table so the sum is exact.  This makes
the program input-independent: no host argsort, no unpermute, and the jitted
shard_map executable is built once per process and cached — warm calls only
transfer inputs, run, and fetch outputs.

The dominant cost is the ~50-70 MB/s axon tunnel.  The [48, 2M] f32 output
(384 MB, ~5.5 s) is therefore returned as int8 with a per-(comp row,
512-point half-group) scale and dequantized on the host; the f32 scales ride
along in the last 1960 columns of the one int8 output tensor (one fetch, ~96
MB total).  Quantization is uniform with step absmax_row/126.5, so the error
is <= absmax/253 (~0.4% of global absmax), far under the 2e-2 gate.  Coords
are shipped as int16 (round(x*32767), 12 MB instead of 24 MB; adds ~0.4% more
error, still 2x under the gate) and widened on device.  Donated output
buffers are created device-side (no 100-400 MB host->device zeros).  Output
shards are fetched one at a time while worker threads dequantize the
previous core's block.  A 7-bit packed variant (8 values -> 7 bytes) was
measured: the 12% byte cut is swamped by tunnel variance and the host-side
unpack cost; int8 wins.

The call runs as N_HALF=2 sequential executions of one half-size program:
the tunnel is duplex enough (measured: a concurrent 12 MB upload adds ~0.2 s
to a 96 MB fetch instead of ~0.4 s serial) that half 2's coordinate upload,
zero-buffer memset, and exec all hide under half 1's output fetch, and the
first fetch starts ~0.2 s earlier.  Going to 4 splits is a wash: each extra
split adds ~8 per-shard fetch round-trips (~10 ms each) for less head saving.

Device pipeline per group (1024 pts = 2 column-tiles of 512 packed into
psum rows [0:64) and [64:128)):
  PE:   broadcast coord row -> psum bc [128, 1024] (K=1 matmul with ones)
        per chunk c: v matmuls [48->64, 512] accumulate into vp psum
  ACT:  t_c = |149.5*x + (149.5 - 127c - lane)|   (abs pass, psum -> sbuf)
  DVE:  e'_c = min(t_c, 1) - 1  (= -relu(1-|.|); tables are negated)
        out = v0 * v1 * v2, absmax-reduce, reciprocal, int8 quantize
  DMA:  out tile [48, 512] x2 -> HBM int8, per-group scales at the end
"""

import sys

sys.path.insert(0, "/opt/trn_rl_repo")

import contextlib
import os

os.environ.setdefault("JAX_PLATFORMS", "axon,cpu")

import numpy as np

import concourse.bass as bass
import concourse.mybir as mybir
from concourse import tile

F32 = mybir.dt.float32
I8 = mybir.dt.int8
I16 = mybir.dt.int16
AF = mybir.ActivationFunctionType
ALU = mybir.AluOpType

N_COMP = 48
G = 300
N_CORES = 8
TILE = 512
GROUP = 2 * TILE  # 1024 points per device group
N_PTS = 2_000_000
NPC = N_PTS // N_CORES  # 250_000 points per core
# The work is split into N_HALF sequential executions of one half-size
# program: the axon tunnel is duplex enough that half 2's coord upload and
# exec hide under half 1's output fetch.
N_HALF = 2
NPC_H = NPC // N_HALF  # 125_000 points per core per half
N_GROUPS = -(-NPC_H // GROUP)  # 123
NPAD = N_GROUPS * GROUP  # 125_952
SLAB = 8  # groups of coords per load slab
QMAX = 126.5  # quant range; <127 so rounding can't overflow int8
CSCALE = 32767.0  # coords are shipped as int16 = round(x * CSCALE)
# the [2*48, N_GROUPS] f32 scales ride along in the int8 output tensor:
# 2*48*N_GROUPS*4 bytes spread over 48 rows of extra int8 columns
SCL_COLS = 2 * N_GROUPS * 4  # 984
NCOL = NPAD + SCL_COLS


def _legalize_sync_waits(nc, max_waits=1):
    """This walrus build accepts at most one sync-wait per instruction; split
    extra waits onto preceding same-engine drains (same-queue => in order)."""
    n = 0
    for f in nc.m.functions:
        for bb in f.blocks:
            new_list = []
            for ins in bb.instructions:
                si = ins.sync_info
                waits = list(si.on_wait) if si and si.on_wait else []
                if len(waits) > max_waits:
                    head, tail = waits[:-max_waits], waits[-max_waits:]
                    for w in head:
                        n += 1
                        import bass_rust as _br
                        new_list.append(
                            _br.InstNoOp(
                                name=f"{ins.name}-wsplit-{n}",
                                engine=ins.engine,
                                ins=[],
                                outs=[],
                                sync_info=mybir.SyncInfo(on_wait=[w], on_update=[]),
                            )
                        )
                    ins.sync_info = mybir.SyncInfo(
                        on_wait=tail,
                        on_update=list(si.on_update) if si.on_update else [],
                    )
                new_list.append(ins)
            bb.instructions[:] = new_list
    return n


def _build_program():
    nc = bass.Bass("TRN2", target_bir_lowering=False, debug=False, num_devices=N_CORES)
    d_coords = nc.dram_tensor("coords", [3, NPAD], I16, kind="ExternalInput")
    d_lhsT = nc.dram_tensor("lhsT", [9, 128, 64], F32, kind="ExternalInput")
    d_bias = nc.dram_tensor("bias", [128, 3], F32, kind="ExternalInput")
    d_outq = nc.dram_tensor("outq", [N_COMP, NCOL], I8, kind="ExternalOutput")

    with tile.TileContext(nc) as tc:
        with contextlib.ExitStack() as ctx:
            const = ctx.enter_context(tc.tile_pool(name="const", bufs=1))
            slabp = ctx.enter_context(tc.tile_pool(name="slabp", bufs=2))
            work = ctx.enter_context(tc.tile_pool(name="work", bufs=2))
            outp = ctx.enter_context(tc.tile_pool(name="outp", bufs=3))
            qp = ctx.enter_context(tc.tile_pool(name="qp", bufs=3))
            bcp = ctx.enter_context(tc.tile_pool(name="bcp", bufs=1, space="PSUM"))
            vpp = ctx.enter_context(tc.tile_pool(name="vpp", bufs=6, space="PSUM"))

            lhsT = const.tile([128, 9 * 64], F32)
            nc.sync.dma_start(
                lhsT[:].rearrange("p (n d) -> p n d", d=64),
                d_lhsT.ap().rearrange("n p d -> p n d"),
            )
            biast = const.tile([128, 3], F32)
            nc.sync.dma_start(biast[:], d_bias.ap())
            onest = const.tile([65, 128], F32)
            for a in range(3):
                nc.vector.memset(onest[32 * a : 32 * a + 1, :], 1.0)
            scl = const.tile([128, N_GROUPS], F32)

            conv = None
            for g in range(N_GROUPS):
                s = g % SLAB
                if s == 0:
                    ncols = min(SLAB * GROUP, NPAD - g * GROUP)
                    slab = slabp.tile([65, SLAB * GROUP], I16, name="slab", tag="slab")
                    for a in range(3):
                        nc.sync.dma_start(
                            slab[32 * a : 32 * a + 1, 0:ncols],
                            d_coords.ap()[a : a + 1, g * GROUP : g * GROUP + ncols],
                        )
                    conv = slabp.tile([65, SLAB * GROUP], F32, name="conv", tag="conv")
                    for a in range(3):
                        nc.vector.tensor_copy(
                            conv[32 * a : 32 * a + 1, 0:ncols],
                            slab[32 * a : 32 * a + 1, 0:ncols],
                        )
                vps = []
                for a in range(3):
                    crow = conv[32 * a : 32 * a + 1, s * GROUP : (s + 1) * GROUP]
                    bc = bcp.tile([128, GROUP], F32, name=f"bc_{g}_{a}", tag="bc")
                    nc.tensor.matmul(
                        bc[:, 0:TILE], onest[32 * a : 32 * a + 1, :], crow[:, 0:TILE],
                        start=True, stop=True,
                    )
                    nc.tensor.matmul(
                        bc[:, TILE:GROUP], onest[32 * a : 32 * a + 1, :], crow[:, TILE:GROUP],
                        start=True, stop=True,
                    )
                    vp = vpp.tile([128, TILE], F32, name=f"vp_{g}_{a}", tag="vp")
                    for c in range(3):
                        tabs = work.tile(
                            [128, GROUP], F32, name=f"tabs_{g}_{a}_{c}", tag="tabs", bufs=3
                        )
                        nc.scalar.activation(
                            tabs[:], bc[:], AF.Abs, bias=biast[:, c : c + 1],
                            scale=float(149.5 / CSCALE),
                        )
                        eneg = work.tile(
                            [128, GROUP], F32, name=f"eneg_{g}_{a}_{c}", tag="eneg", bufs=3
                        )
                        nc.vector.tensor_scalar(
                            eneg[:], tabs[:], 1.0, 1.0, ALU.min, ALU.subtract
                        )
                        lt = lhsT[:, (a * 3 + c) * 64 : (a * 3 + c + 1) * 64]
                        nc.tensor.matmul(
                            vp[0:64, :], lt, eneg[:, 0:TILE],
                            start=(c == 0), stop=(c == 2), tile_position=(0, 0),
                        )
                        nc.tensor.matmul(
                            vp[64:128, :], lt, eneg[:, TILE:GROUP],
                            start=(c == 0), stop=(c == 2), tile_position=(0, 64),
                        )
                    vps.append(vp)

                v1sb = outp.tile([128, TILE], F32, name=f"v1sb_{g}", tag="v1sb")
                nc.vector.tensor_copy(v1sb[:], vps[1][:])
                p01 = outp.tile([128, TILE], F32, name=f"p01_{g}", tag="p01")
                nc.vector.tensor_mul(p01[:], vps[0][:], v1sb[:])
                outt = outp.tile([128, TILE], F32, name=f"outt_{g}", tag="outt")
                nc.vector.tensor_mul(outt[:], vps[2][:], p01[:])

                nc.vector.tensor_reduce(
                    scl[:, g : g + 1], outt[:], axis=mybir.AxisListType.X,
                    op=ALU.max, apply_absolute_value=True,
                )
                clamped = qp.tile([128, 1], F32, name=f"cl_{g}", tag="cl")
                nc.vector.tensor_scalar_max(clamped[:], scl[:, g : g + 1], 1e-12)
                rcp = qp.tile([128, 1], F32, name=f"rcp_{g}", tag="rcp")
                nc.vector.reciprocal(rcp[:], clamped[:])
                outq = qp.tile([128, TILE], I8, name=f"outq_{g}", tag="outq")
                nc.vector.tensor_scalar(
                    outq[:], outt[:], rcp[:, 0:1], QMAX, ALU.mult, ALU.mult
                )

                off = g * GROUP
                nc.sync.dma_start(d_outq.ap()[:, off : off + TILE], outq[0:N_COMP, :])
                nc.sync.dma_start(
                    d_outq.ap()[:, off + TILE : off + GROUP], outq[64 : 64 + N_COMP, :]
                )

            # scales ride in the last SCL_COLS int8 columns: rows 0:48 are the
            # half-A scales ([48, N_GROUPS] f32 = [48, 4*N_GROUPS] bytes),
            # rows 64:112 the half-B scales
            half = 4 * N_GROUPS
            sclb = scl[:].bitcast(I8)  # [128, 4*N_GROUPS]
            nc.sync.dma_start(
                d_outq.ap()[:, NPAD : NPAD + half], sclb[0:N_COMP, :]
            )
            nc.sync.dma_start(
                d_outq.ap()[:, NPAD + half : NPAD + 2 * half], sclb[64 : 64 + N_COMP, :]
            )

    from concourse.hw_specs import get_activation_tables
    import bass_rust as _br
    _br.insert_act_table_loads(nc, list(get_activation_tables(nc.m.arch).items()))
    _legalize_sync_waits(nc)
    return nc


_RT: dict = {}
_OUT_BUF = None
_TABLES = None
_CH_BUF: list = []  # reused int16 coordinate staging buffers, one per half
# Device output buffers from the previous call, reused as the donated
# "zero" operands of the next call. The program writes every byte of the
# output tensor (all point columns via the group loop, all scale columns via
# the two trailing DMAs), so zero-init is only a first-call formality and
# skipping the ~50 ms memset+dispatch takes it off the exec critical path.
_PREV: list = []


def _runtime():
    """Build the Bass program and the jitted shard_map executable once."""
    if _RT:
        return _RT
    import jax
    import jax.numpy as jnp
    from jax.experimental.shard_map import shard_map
    from jax.sharding import Mesh, NamedSharding, PartitionSpec as P

    from concourse.bass2jax import (
        _bass_exec_p,
        install_neuronx_cc_hook,
        partition_id_tensor,
    )

    install_neuronx_cc_hook()
    nc = _build_program()

    partition_name = nc.partition_id_tensor.name if nc.partition_id_tensor else None
    in_names, out_names, out_avals = [], [], []
    for alloc in nc.m.functions[0].allocations:
        if not isinstance(alloc, mybir.MemoryLocationSet):
            continue
        name = alloc.memorylocations[0].name
        if alloc.kind == "ExternalInput":
            if name != partition_name:
                in_names.append(name)
        elif alloc.kind == "ExternalOutput":
            out_names.append(name)
            out_avals.append(
                jax.core.ShapedArray(tuple(alloc.tensor_shape), mybir.dt.np(alloc.dtype))
            )
    assert in_names == ["coords", "lhsT", "bias"], in_names
    assert out_names == ["outq"], out_names
    n_params = len(in_names)
    n_outs = len(out_names)
    all_names = in_names + out_names
    if partition_name is not None:
        all_names.append(partition_name)
    all_names = tuple(all_names)

    def _body(*args):
        operands = list(args)
        if partition_name is not None:
            operands.append(partition_id_tensor())
        outs = _bass_exec_p.bind(
            *operands,
            out_avals=tuple(out_avals),
            in_names=all_names,
            out_names=tuple(out_names),
            lowering_input_output_aliases=(),
            sim_require_finite=True,
            sim_require_nnan=True,
            nc=nc,
        )
        return tuple(outs)

    devices = jax.devices()[:N_CORES]
    assert len(devices) == N_CORES
    mesh = Mesh(np.asarray(devices), ("core",))
    sh = NamedSharding(mesh, P("core"))
    donate = tuple(range(n_params, n_params + n_outs))
    sharded = jax.jit(
        shard_map(
            _body,
            mesh=mesh,
            in_specs=(P("core"),) * (n_params + n_outs),
            out_specs=(P("core"),) * n_outs,
            check_rep=False,
        ),
        donate_argnums=donate,
        keep_unused=True,
    )

    # both halves' donated zero buffers in one dispatch
    zeros = jax.jit(
        lambda: tuple(
            jnp.zeros((N_CORES * N_COMP, NCOL), jnp.int8) for _ in range(N_HALF)
        ),
        out_shardings=(sh,) * N_HALF,
    )

    _RT.update(sharded=sharded, zeros=zeros, sh=sh)
    return _RT


def _build_tables(params):
    """lhsT[a*3+c] = -param_a[:, 127c : 127c+128].T zero-padded to [128, 64].
    Lane 0 of chunks 1,2 duplicates lane 127 of the previous chunk (grid rows
    127, 254) — zero it there so summing all three chunk products is exact."""
    lhsT9 = np.zeros((9, 128, 64), dtype=np.float32)
    for a in range(3):
        for c in range(3):
            rows = params[a][:, 127 * c : 127 * c + 128]
            lhsT9[a * 3 + c, : rows.shape[1], :N_COMP] = -rows.T
            if c > 0:
                lhsT9[a * 3 + c, 0, :] = 0.0
    bias = np.zeros((128, 3), dtype=np.float32)
    for c in range(3):
        bias[:, c] = 149.5 - 127.0 * c - np.arange(128)
    return lhsT9, bias


def _dequant_core(k, h, qk, out):
    """Dequantize core k / half h's int8 block [48, NCOL] into
    out[:, k*NPC + h*NPC_H : ...]."""
    ngf = NPC_H // GROUP  # 122 full groups per half
    full = ngf * GROUP
    tail = NPC_H - full
    strided = np.lib.stride_tricks.as_strided
    shalf = 4 * N_GROUPS
    inv = np.float32(1.0 / QMAX)
    sA = qk[:, NPAD : NPAD + shalf].copy().view(np.float32) * inv  # [48, N_GROUPS]
    sB = qk[:, NPAD + shalf :].copy().view(np.float32) * inv
    # scale per (comp, group, half-tile): [48, ngf, 2, 1]
    s3 = np.stack([sA[:, :ngf], sB[:, :ngf]], axis=2)[..., None]
    base = out[:, k * NPC + h * NPC_H :]
    B = strided(base, shape=(N_COMP, ngf, 2, TILE),
                strides=(out.strides[0], GROUP * 4, TILE * 4, 4))
    Q = strided(qk, shape=(N_COMP, ngf, 2, TILE),
                strides=(qk.strides[0], GROUP, TILE, 1))
    np.multiply(Q, s3, out=B)
    if tail:
        np.multiply(qk[:, full : full + tail], sA[:, ngf : ngf + 1],
                    out=base[:, full:NPC_H])


def kernel(xyz_sampled, param0, param1, param2):
    from concurrent.futures import ThreadPoolExecutor

    xyz = np.asarray(xyz_sampled, dtype=np.float32)
    params = [
        np.ascontiguousarray(p.reshape(p.shape[1], p.shape[2]), dtype=np.float32)
        for p in (param0, param1, param2)
    ]
    n = xyz.shape[0]
    assert n == N_PTS and n % N_CORES == 0

    import jax

    rt = _runtime()

    # The tables derive from the tiny params (172 KB) — cache the device
    # arrays keyed on a content hash so identical warm calls skip the rebuild
    # and the ~2.4 MB upload on the exec critical path.
    import hashlib

    global _TABLES
    key = hashlib.blake2b(
        b"".join(p.tobytes() for p in params), digest_size=16
    ).digest()
    if _TABLES is None or _TABLES[0] != key:
        lhsT9, bias = _build_tables(params)
        lhsT_d = jax.device_put(np.tile(lhsT9, (N_CORES, 1, 1)), rt["sh"])
        bias_d = jax.device_put(np.tile(bias, (N_CORES, 1)), rt["sh"])
        _TABLES = (key, lhsT_d, bias_d)
    else:
        _, lhsT_d, bias_d = _TABLES

    global _PREV
    if len(_PREV) == N_HALF:
        zs, _PREV = _PREV, []
    else:
        zs = rt["zeros"]()
    xr = xyz.reshape(N_CORES, N_HALF, NPC_H, 3)
    # dispatch halves back-to-back: half h+1's coord prep and upload plus its
    # exec all overlap half h's upload/exec/output-fetch (the tunnel is
    # duplex); quantize each half's coords just before its dispatch so the
    # later halves' host prep also hides under earlier transfers
    outs = []
    coords = []  # keep buffers alive until fetch completes
    for h in range(N_HALF):
        if len(_CH_BUF) <= h:
            _CH_BUF.append(np.empty((N_CORES, 3, NPAD), dtype=np.int16))
        ch = _CH_BUF[h]
        ch[:, :, NPC_H:] = 0  # pad coords (any in-range value works)
        ch[:, :, :NPC_H] = np.rint(
            xr[:, h] * np.float32(CSCALE)
        ).astype(np.int16).transpose(0, 2, 1)
        coords.append(ch)
        (o,) = rt["sharded"](ch.reshape(N_CORES * 3, NPAD), lhsT_d, bias_d, zs[h])
        outs.append(o)

    shardlists = []
    for o in outs:
        sl = sorted(
            (sd.index[0].start // N_COMP, sd.data) for sd in o.addressable_shards
        )
        for _, d in sl:
            d.copy_to_host_async()
        shardlists.append(sl)

    # Reuse the 384 MB result buffer across calls: a fresh np.empty is backed
    # by new mmap pages, and first-touch page faults during dequant cost
    # ~0.1 s per call. Every byte is rewritten below before return.
    global _OUT_BUF
    if _OUT_BUF is None or _OUT_BUF.shape != (N_COMP, n):
        _OUT_BUF = np.empty((N_COMP, n), dtype=np.float32)
    out = _OUT_BUF
    # Fetch per-shard over the (serial) tunnel; dequantize each core's block
    # in a worker thread while the next shard transfers.
    with ThreadPoolExecutor(4) as ex:
        futs = []
        for h, sl in enumerate(shardlists):
            for k, d in sl:
                qk = np.asarray(d)  # [48, NCOL] int8
                futs.append(ex.submit(_dequant_core, k, h, qk, out))
        for f in futs:
            f.result()
    _PREV = outs  # fully fetched; donate these buffers on the next call
    return out


if __name__ == "__main__":
    # quick self-test against numpy reference on the full-size random input
    rng = np.random.default_rng(0)
    xyz = rng.uniform(-1, 1, size=(N_PTS, 3)).astype(np.float32)
    ps = [0.2 * rng.standard_normal((1, N_COMP, G, 1)).astype(np.float32) for _ in range(3)]

    def ref_interp(p, coord):
        pp = p[0, :, :, 0]
        pos = (coord + 1.0) * 0.5 * (G - 1)
        i0 = np.clip(np.floor(pos).astype(np.int64), 0, G - 1)
        i1 = np.minimum(i0 + 1, G - 1)
        w = (pos - i0).astype(np.float32)
        return pp[:, i0] * (1.0 - w) + pp[:, i1] * w

    got = kernel(xyz, *ps)
    exp = ref_interp(ps[0], xyz[:, 0]) * ref_interp(ps[1], xyz[:, 1]) * ref_interp(ps[2], xyz[:, 2])
    err = np.abs(got - exp).max()
    print("max abs err:", err, "absmax:", np.abs(exp).max(), "rel:", err / np.abs(exp).max())
